# revision 1
# baseline (speedup 1.0000x reference)
"""Davies-Bouldin index (segment_reduce) Trainium2 kernel.

Strategy (one pass over the data instead of the reference's two):
  segsum(|x - A_c|^2)[k] = Q_k - 2*A_k.S_k + n_k*|A_k|^2
with S_k = segsum(x), Q_k = segsum(|x|^2), n_k = counts. The device computes
only S and the per-dim square sums S2 (Q = S2.sum(-1)) via a one-hot matmul
over bf16 data, data-parallel over 8 NeuronCores; counts and all K-sized
math run on the host in fp64.

Device per 128-point block b of a supertile:
  H_b[p,k]  = (cls[p,b] == k)        (DVE tensor_scalar is_equal vs iota row)
  psum     += H_b^T @ [X_b | X_b^2]  (PE, fp32 PSUM accumulation)
X^2 is computed by the ACT engine (Square). Per-core output is [128, 128]
fp32: cols 0:64 = S rows, cols 64:128 = S2 rows; rows 100..127 are padding.
"""

from contextlib import ExitStack

import numpy as np
import ml_dtypes

# ---- hardcoded problem geometry (nn_DBI_44985487458968) ----
N_TOTAL = 2_000_000
D = 64
K = 100
N_CORES = 8
P = 128
KPAD = 128            # (legacy) padded one-hot width
KP = 100              # one-hot width = K (no FWL in this toolchain, no pad)
B = 32                # 128-point blocks per supertile
SUP = P * B           # 4096 points per supertile
PER_CORE = N_TOTAL // N_CORES          # 250_000
NSUP = -(-PER_CORE // SUP)             # 62 supertiles
PADN = NSUP * SUP                      # 253_952 padded points per core
PAD_CLS = 127          # pad points land in ignored one-hot column 127

BF16 = ml_dtypes.bfloat16


def _split_excess_waits(nc):
    """Walrus allows one semaphore wait per instruction (two on
    EventSemaphore). Tile's tail drain aggregates one wait per live proc,
    which this compiler build rejects — hoist the extras into standalone
    NoOp wait-carriers executed just before, same engine, same semantics."""
    import concourse.mybir as mybir

    for bb in nc.main_func.blocks:
        new = []
        for inst in bb.instructions:
            si = inst.sync_info
            limit = 2 if isinstance(inst, mybir.InstEventSemaphore) else 1
            if si is not None and si.on_wait and len(si.on_wait) > limit:
                waits = list(si.on_wait)
                for w in waits[:-limit]:
                    nop = mybir.InstNoOp(
                        name=nc.get_next_instruction_name(),
                        engine=inst.engine,
                        ins=[], outs=[],
                        sync_info=mybir.SyncInfo(on_wait=[w], on_update=[]),
                    )
                    nc.register_instruction(nop)
                    new.append(nop)
                inst.sync_info = mybir.SyncInfo(
                    on_wait=waits[-limit:], on_update=list(si.on_update))
            new.append(inst)
        bb.instructions[:] = new


def _build_module(nsup: int, b: int):
    import concourse.bass as bass
    import concourse.mybir as mybir
    import concourse.tile as tile

    sup_cols = b * D                      # X columns per supertile
    nc = bass.Bass()
    x_in = nc.dram_tensor("x", [nsup, P, sup_cols], mybir.dt.bfloat16,
                          kind="ExternalInput")
    cls_in = nc.dram_tensor("cls", [P, nsup * b], mybir.dt.bfloat16,
                            kind="ExternalInput")
    # iota_kb[p, k*b + j] = k, so one tensor_tensor is_equal against a
    # broadcast cls slice emits the whole supertile's one-hot at 2x mode.
    iota_in = nc.dram_tensor("iota", [P, KP * b], mybir.dt.bfloat16,
                             kind="ExternalInput")
    out = nc.dram_tensor("out", [KP, 2 * D], mybir.dt.float32,
                         kind="ExternalOutput")

    n_mm_total = nsup * b
    with ExitStack() as ctx:
        tc = ctx.enter_context(tile.TileContext(nc))
        cpool = ctx.enter_context(tc.tile_pool(name="const", bufs=1))
        xpool = ctx.enter_context(tc.tile_pool(name="x", bufs=3))
        hpool = ctx.enter_context(tc.tile_pool(name="h", bufs=3))
        ppool = ctx.enter_context(tc.tile_pool(name="psum", bufs=1, space="PSUM"))
        opool = ctx.enter_context(tc.tile_pool(name="o", bufs=1))

        iota_t = cpool.tile([P, KP * b], mybir.dt.bfloat16)
        nc.sync.dma_start(out=iota_t[:], in_=iota_in[:])
        cls_t = cpool.tile([P, nsup * b], mybir.dt.bfloat16)
        nc.sync.dma_start(out=cls_t[:], in_=cls_in[:])

        psum_t = ppool.tile([P, 2 * D], mybir.dt.float32)

        n_mm = 0
        for s in range(nsup):
            xb = xpool.tile([P, 2 * sup_cols], mybir.dt.bfloat16)
            nc.sync.dma_start(out=xb[:, 0:sup_cols], in_=x_in[s])
            nc.scalar.activation(
                out=xb[:, sup_cols:2 * sup_cols],
                in_=xb[:, 0:sup_cols],
                func=mybir.ActivationFunctionType.Square,
            )
            # One-hot for the whole supertile in one DVE op (2x mode):
            # G[p, k*b + j] = (cls[p, s*b+j] == k).
            gt = hpool.tile([P, KP * b], mybir.dt.bfloat16)
            cls_bc = cls_t[:, s * b:(s + 1) * b].unsqueeze(1).broadcast_to(
                [P, KP, b])
            nc.vector.tensor_tensor(
                out=gt[:], in0=iota_t[:], in1=cls_bc,
                op=mybir.AluOpType.is_equal,
            )
            gv = gt[:].rearrange("p (k j) -> p k j", j=b)
            xr = xb[:].rearrange("p (two n) -> p two n", two=2)
            for j in range(b):
                nc.tensor.matmul(
                    psum_t[:KP, :],
                    lhsT=gv[:, :, j],
                    rhs=xr[:, :, j * D:(j + 1) * D],
                    start=(n_mm == 0),
                    stop=(n_mm == n_mm_total - 1),
                )
                n_mm += 1

        out_sb = opool.tile([KP, 2 * D], mybir.dt.float32)
        nc.vector.tensor_copy(out=out_sb[:], in_=psum_t[:KP, :])
        nc.sync.dma_start(out=out[:], in_=out_sb[:])
    _split_excess_waits(nc)
    return nc


def _prep_core_inputs(x_shard: np.ndarray, cls_shard: np.ndarray,
                      nsup: int, b: int) -> dict:
    """Pad + lay out one core's shard for the device kernel."""
    sup = P * b
    padn = nsup * sup
    npts = x_shard.shape[0]
    xb16 = np.zeros((padn, D), dtype=BF16)
    xb16[:npts] = x_shard.astype(BF16)
    clsf = np.full((padn,), PAD_CLS, dtype=BF16)
    clsf[:npts] = cls_shard.astype(BF16)
    # xb[s, p, j*D+d] = x[s*sup + p*b + j, d]
    x_dev = np.ascontiguousarray(xb16.reshape(nsup, P, b * D))
    # cls_t[p, s*b + j] = cls[s*sup + p*b + j]
    cls_dev = np.ascontiguousarray(
        clsf.reshape(nsup, P, b).transpose(1, 0, 2).reshape(P, nsup * b))
    # iota_kb[p, k*b + j] = k
    iota = np.ascontiguousarray(np.broadcast_to(
        np.repeat(np.arange(KP), b).astype(BF16)[None, :], (P, KP * b)))
    return {"x": x_dev, "cls": cls_dev, "iota": iota}


def _dbi_from_stats(S: np.ndarray, S2: np.ndarray, n: np.ndarray) -> np.float32:
    S = S.astype(np.float64)
    Q = S2.astype(np.float64).sum(-1)
    n = n.astype(np.float64)
    counts = 1.0 + n
    A = (0.001 + S) / counts[:, None]
    segsq = Q - 2.0 * (A * S).sum(-1) + n * (A * A).sum(-1)
    Si = np.sqrt((0.001 + segsq) / counts)
    diff = A[:, None, :] - A[None, :, :]
    sumsq = (diff * diff).sum(-1)
    eye = np.eye(K, dtype=bool)
    Mij = np.sqrt(np.where(eye, 1.0, sumsq))
    Rij = np.where(eye, 0.0, (Si[:, None] + Si[None, :]) / Mij)
    return np.float32(Rij.max(axis=1).sum() / K)


def kernel(data_points: np.ndarray, clustering: np.ndarray) -> np.ndarray:
    from concourse.bass_utils import run_bass_kernel_spmd

    x = np.asarray(data_points)
    cls = np.asarray(clustering)
    assert x.shape == (N_TOTAL, D), x.shape

    nc = _build_module(NSUP, B)
    in_maps = []
    for c in range(N_CORES):
        sl = slice(c * PER_CORE, (c + 1) * PER_CORE)
        in_maps.append(_prep_core_inputs(x[sl], cls[sl], NSUP, B))
    res = run_bass_kernel_spmd(nc, in_maps, core_ids=list(range(N_CORES)))

    S = np.zeros((K, D), np.float64)
    S2 = np.zeros((K, D), np.float64)
    for r in res.results:
        o = r["out"].astype(np.float64)
        S += o[:K, :D]
        S2 += o[:K, D:]
    assert KP >= K
    n = np.bincount(cls.astype(np.int64), minlength=K).astype(np.float64)
    return np.asarray(_dbi_from_stats(S, S2, n), dtype=np.float32)



# revision 5
# speedup vs baseline: 2.5142x; 2.5142x over previous
"""Davies-Bouldin index (segment_reduce) Trainium2 kernel — sorted-block v2.

Host-side: points are sorted by cluster and each cluster is padded with
zero-points to a multiple of 1024 = 8 cores x one 128-row block.  The
1024-point "octets" are dealt round-robin across the 8 cores, so every
core sees the same compile-time sequence of single-cluster 128-point
blocks (the zero pads contribute nothing to any sum).

Device-side (per core, data-parallel): for each block the PE accumulates
  psum[k, (X | X^2)] += W_k^T @ [X_b | X_b^2]
where the stationary weight W_k (column k all-ones, rest zero) only
changes at cluster boundaries (~100 weight loads per core instead of one
per matmul, the baseline's bottleneck).  Blocks are grouped 4 per matmul
(512 moving columns) so the weight load pipelines under the stream.
X^2 is computed on-device (ACT/DVE split).  Weights are built on-device
by two affine_select instructions (an expanded 100x100 eye, broadcast
over partitions).

Host-side epilogue: counts via bincount, and the K x K Davies-Bouldin
reduction in fp64 from the device partials S = segsum(x), S2 = segsum(x^2)
using  segsum(|x - A_c|^2) = Q - 2*A.S + n*|A|^2,  Q = S2.sum(-1).
"""

from contextlib import ExitStack

import numpy as np
import ml_dtypes

# ---- hardcoded problem geometry (nn_DBI_44985487458968) ----
N_TOTAL = 2_000_000
D = 64
K = 100
N_CORES = 8
P = 128
OCT = P * N_CORES      # cluster padding quantum: 1024 points
B = 64                 # blocks per supertile -> 1 MiB DMA chunks
SUPC = B * D           # X columns per supertile
W0K = 20               # clusters [0, W0K) live in the early weight tile

BF16 = ml_dtypes.bfloat16


def _split_excess_waits(nc):
    """Walrus allows one semaphore wait per instruction (two on
    EventSemaphore). Tile's tail drain aggregates one wait per live proc,
    which this compiler build rejects — hoist the extras into standalone
    NoOp wait-carriers executed just before, same engine, same semantics."""
    import concourse.mybir as mybir

    for bb in nc.main_func.blocks:
        new = []
        for inst in bb.instructions:
            si = inst.sync_info
            limit = 2 if isinstance(inst, mybir.InstEventSemaphore) else 1
            if si is not None and si.on_wait and len(si.on_wait) > limit:
                waits = list(si.on_wait)
                for w in waits[:-limit]:
                    nop = mybir.InstNoOp(
                        name=nc.get_next_instruction_name(),
                        engine=inst.engine,
                        ins=[], outs=[],
                        sync_info=mybir.SyncInfo(on_wait=[w], on_update=[]),
                    )
                    nc.register_instruction(nop)
                    new.append(nop)
                inst.sync_info = mybir.SyncInfo(
                    on_wait=waits[-limit:], on_update=list(si.on_update))
            new.append(inst)
        bb.instructions[:] = new


def _plan(cls):
    """Cluster counts and the shared per-core block -> cluster schedule."""
    counts = np.bincount(cls, minlength=K)
    m = -(-counts // OCT)            # octets per cluster
    m = np.maximum(m, 1)
    noct = int(m.sum())              # per-core real blocks
    nsup = -(-noct // B)
    blocks_cluster = np.repeat(np.arange(K), m)
    return counts, m, noct, nsup, blocks_cluster


def _chunks(blocks_cluster, nsup):
    """MM chunks (supertile, first block in supertile, nblocks<=4, cluster)."""
    chunks = []
    nb = len(blocks_cluster)
    for s in range(nsup):
        j = 0
        while j < B:
            g0 = s * B + j
            if g0 >= nb:
                break
            k = int(blocks_cluster[g0])
            run = 1
            while j + run < B and g0 + run < nb and blocks_cluster[g0 + run] == k:
                run += 1
            off = 0
            while off < run:
                g = min(4, run - off)
                chunks.append((s, j + off, g, k))
                off += g
            j += run
    return chunks


def _build_module(nsup, chunks, blocks_cluster):
    import concourse.bass as bass
    import concourse.mybir as mybir
    import concourse.tile as tile

    nc = bass.Bass()
    x_in = nc.dram_tensor("x", [nsup, P, SUPC], mybir.dt.bfloat16,
                          kind="ExternalInput")
    out = nc.dram_tensor("out", [K, 2 * D], mybir.dt.float32,
                         kind="ExternalOutput")

    n_mm = len(chunks)
    per_s = {}
    for ch in chunks:
        per_s.setdefault(ch[0], []).append(ch)

    # first supertile that references a cluster >= W0K decides when the
    # big weight tile must be ready; build it early only if needed early.
    first_hi = next((i for i, k in enumerate(blocks_cluster) if k >= W0K),
                    len(blocks_cluster))
    w1_emit_s = max(0, min(first_hi // B - 2, 2))

    with ExitStack() as ctx:
        tc = ctx.enter_context(tile.TileContext(nc))
        cpool = ctx.enter_context(tc.tile_pool(name="const", bufs=1))
        xpool = ctx.enter_context(tc.tile_pool(name="x", bufs=3))
        ppool = ctx.enter_context(tc.tile_pool(name="psum", bufs=1, space="PSUM"))
        opool = ctx.enter_context(tc.tile_pool(name="o", bufs=1))

        ones_t = cpool.tile([P, 1], mybir.dt.bfloat16, tag="ones")
        nc.vector.memset(ones_t[:], 1.0)
        w0 = cpool.tile([P, W0K * K], mybir.dt.bfloat16, tag="w0")
        w1 = cpool.tile([P, (K - W0K) * K], mybir.dt.bfloat16, tag="w1")

        def build_w(wt, nk, base):
            nc.gpsimd.affine_select(
                out=wt[:],
                in_=ones_t[:, 0:1].broadcast_to([P, nk * K]),
                pattern=[[-1, nk], [1, K]], base=base, channel_multiplier=0,
                compare_op=mybir.AluOpType.is_equal, fill=0.0)

        build_w(w0, W0K, 0)
        w1_built = False
        if w1_emit_s == 0:
            build_w(w1, K - W0K, -W0K)
            w1_built = True

        # All matmuls run with start=False and accumulate onto this explicit
        # zero: correct whether PSUM has_written bits are stale (accumulate
        # onto 0) or clear (overwrite with the product) — and it keeps slots
        # never touched by a partial-width chunk at exactly 0 for the fold.
        psum_t = ppool.tile([P, 8 * D], mybir.dt.float32)    # [128, 512]
        nc.vector.memset(psum_t[:K, :], 0.0)

        mm_i = 0
        for s in range(nsup):
            xb = xpool.tile([P, 2 * SUPC], mybir.dt.bfloat16)
            nc.sync.dma_start(out=xb[:, 0:SUPC], in_=x_in[s])
            if s % 3 == 0:
                nc.scalar.activation(
                    out=xb[:, SUPC:2 * SUPC], in_=xb[:, 0:SUPC],
                    func=mybir.ActivationFunctionType.Square)
            else:
                nc.vector.tensor_tensor(
                    out=xb[:, SUPC:2 * SUPC], in0=xb[:, 0:SUPC],
                    in1=xb[:, 0:SUPC], op=mybir.AluOpType.mult)
            if not w1_built and s >= w1_emit_s:
                build_w(w1, K - W0K, -W0K)
                w1_built = True
            for (_, j0, g, k) in per_s.get(s, []):
                lhsT = (w0[:, k * K:(k + 1) * K] if k < W0K
                        else w1[:, (k - W0K) * K:(k - W0K + 1) * K])
                for half in range(2):
                    nc.tensor.matmul(
                        psum_t[:K, half * 4 * D:half * 4 * D + g * D],
                        lhsT=lhsT,
                        rhs=xb[:, half * SUPC + j0 * D:
                               half * SUPC + (j0 + g) * D],
                        start=False, stop=(mm_i == 2 * n_mm - 1),
                        skip_group_check=True)
                    mm_i += 1
        if not w1_built:
            build_w(w1, K - W0K, -W0K)

        # fold the 4 block-slots of each half: S = sum_j A[:, j*64:(j+1)*64]
        tmp = opool.tile([P, 8 * D + 4 * D], mybir.dt.float32, tag="tmp")
        out_sb = opool.tile([P, 2 * D], mybir.dt.float32, tag="osb")
        nc.vector.tensor_copy(out=tmp[:K, 0:8 * D], in_=psum_t[:K, :])
        for h in range(2):
            b0 = h * 4 * D
            sc = 8 * D + h * 2 * D
            nc.vector.tensor_tensor(
                out=tmp[:K, sc:sc + D], in0=tmp[:K, b0:b0 + D],
                in1=tmp[:K, b0 + D:b0 + 2 * D], op=mybir.AluOpType.add)
            nc.vector.tensor_tensor(
                out=tmp[:K, sc + D:sc + 2 * D], in0=tmp[:K, b0 + 2 * D:b0 + 3 * D],
                in1=tmp[:K, b0 + 3 * D:b0 + 4 * D], op=mybir.AluOpType.add)
            nc.vector.tensor_tensor(
                out=out_sb[:K, h * D:(h + 1) * D], in0=tmp[:K, sc:sc + D],
                in1=tmp[:K, sc + D:sc + 2 * D], op=mybir.AluOpType.add)
        nc.sync.dma_start(out=out[:], in_=out_sb[:K, :])
    _split_excess_waits(nc)
    return nc


def _prep_inputs(x, cls, counts, m, noct, nsup):
    """Sorted/padded per-core device arrays [nsup, P, SUPC] bf16."""
    n = x.shape[0]
    order = np.argsort(cls, kind="stable")
    sel = np.full(noct * OCT, n, np.int64)       # n -> zero row
    in_off = 0
    out_off = 0
    for k in range(K):
        nk = int(counts[k])
        sel[out_off:out_off + nk] = order[in_off:in_off + nk]
        in_off += nk
        out_off += int(m[k]) * OCT
    x_ext = np.zeros((n + 1, D), BF16)
    x_ext[:n] = x
    xg = x_ext[sel].reshape(noct, N_CORES, P, D)
    nbp = nsup * B
    in_maps = []
    for c in range(N_CORES):
        xc = np.zeros((nbp, P, D), BF16)
        xc[:noct] = xg[:, c]
        xdev = np.ascontiguousarray(
            xc.reshape(nsup, B, P, D).transpose(0, 2, 1, 3).reshape(
                nsup, P, SUPC))
        in_maps.append({"x": xdev})
    return in_maps


def prepare(x, cls):
    """Build (module, per-core inputs, counts) for the full problem."""
    counts, m, noct, nsup, blocks_cluster = _plan(cls)
    chunks = _chunks(blocks_cluster, nsup)
    nc = _build_module(nsup, chunks, blocks_cluster)
    in_maps = _prep_inputs(x, cls, counts, m, noct, nsup)
    return nc, in_maps, counts


def _dbi_from_stats(S, S2, n):
    S = S.astype(np.float64)
    Q = S2.astype(np.float64).sum(-1)
    n = n.astype(np.float64)
    counts = 1.0 + n
    A = (0.001 + S) / counts[:, None]
    segsq = Q - 2.0 * (A * S).sum(-1) + n * (A * A).sum(-1)
    Si = np.sqrt((0.001 + segsq) / counts)
    diff = A[:, None, :] - A[None, :, :]
    sumsq = (diff * diff).sum(-1)
    eye = np.eye(K, dtype=bool)
    Mij = np.sqrt(np.where(eye, 1.0, sumsq))
    Rij = np.where(eye, 0.0, (Si[:, None] + Si[None, :]) / Mij)
    return np.float32(Rij.max(axis=1).sum() / K)


def kernel(data_points: np.ndarray, clustering: np.ndarray) -> np.ndarray:
    from concourse.bass_utils import run_bass_kernel_spmd

    x = np.asarray(data_points)
    cls = np.asarray(clustering).astype(np.int64)
    assert x.shape == (N_TOTAL, D), x.shape

    nc, in_maps, counts = prepare(x, cls)
    res = run_bass_kernel_spmd(nc, in_maps, core_ids=list(range(N_CORES)))

    S = np.zeros((K, D), np.float64)
    S2 = np.zeros((K, D), np.float64)
    for r in res.results:
        o = r["out"].astype(np.float64)
        S += o[:, :D]
        S2 += o[:, D:]
    return np.asarray(_dbi_from_stats(S, S2, counts.astype(np.float64)),
                      dtype=np.float32)


# revision 10
# speedup vs baseline: 2.8582x; 1.1368x over previous
"""Davies-Bouldin index (segment_reduce) Trainium2 kernel — sorted-block v2.

Host-side: points are sorted by cluster and each cluster is padded with
zero-points to a multiple of 1024 = 8 cores x one 128-row block.  The
1024-point "octets" are dealt round-robin across the 8 cores, so every
core sees the same compile-time sequence of single-cluster 128-point
blocks (the zero pads contribute nothing to any sum).

Device-side (per core, data-parallel): for each block the PE accumulates
  psum[k, (X | X^2)] += W_k^T @ [X_b | X_b^2]
where the stationary weight W_k (column k all-ones, rest zero) only
changes at cluster boundaries (~100 weight loads per core instead of one
per matmul, the baseline's bottleneck).  Blocks are grouped 4 per matmul
(512 moving columns) so the weight load pipelines under the stream.
X^2 is computed on-device (ACT/DVE split).  Weights are built on-device
by two affine_select instructions (an expanded 100x100 eye, broadcast
over partitions).

Host-side epilogue: counts via bincount, and the K x K Davies-Bouldin
reduction in fp64 from the device partials S = segsum(x), S2 = segsum(x^2)
using  segsum(|x - A_c|^2) = Q - 2*A.S + n*|A|^2,  Q = S2.sum(-1).
"""

from contextlib import ExitStack

import numpy as np
import ml_dtypes

# ---- hardcoded problem geometry (nn_DBI_44985487458968) ----
N_TOTAL = 2_000_000
D = 64
K = 100
N_CORES = 8
P = 128
OCT = P * N_CORES      # cluster padding quantum: 1024 points
B = 64                 # blocks per supertile -> 1 MiB DMA chunks
SUPC = B * D           # X columns per supertile
W0K = 20               # clusters [0, W0K) live in the early weight tile

BF16 = ml_dtypes.bfloat16


def _split_excess_waits(nc):
    """Walrus allows one semaphore wait per instruction (two on
    EventSemaphore). Tile's tail drain aggregates one wait per live proc,
    which this compiler build rejects — hoist the extras into standalone
    NoOp wait-carriers executed just before, same engine, same semantics."""
    import concourse.mybir as mybir

    for bb in nc.main_func.blocks:
        new = []
        for inst in bb.instructions:
            si = inst.sync_info
            limit = 2 if isinstance(inst, mybir.InstEventSemaphore) else 1
            if si is not None and si.on_wait and len(si.on_wait) > limit:
                waits = list(si.on_wait)
                for w in waits[:-limit]:
                    nop = mybir.InstNoOp(
                        name=nc.get_next_instruction_name(),
                        engine=inst.engine,
                        ins=[], outs=[],
                        sync_info=mybir.SyncInfo(on_wait=[w], on_update=[]),
                    )
                    nc.register_instruction(nop)
                    new.append(nop)
                inst.sync_info = mybir.SyncInfo(
                    on_wait=waits[-limit:], on_update=list(si.on_update))
            new.append(inst)
        bb.instructions[:] = new


def _plan(cls):
    """Cluster counts and the shared per-core block -> cluster schedule."""
    counts = np.bincount(cls, minlength=K)
    m = -(-counts // OCT)            # octets per cluster
    m = np.maximum(m, 1)
    noct = int(m.sum())              # per-core real blocks
    nsup = -(-noct // B)
    blocks_cluster = np.repeat(np.arange(K), m)
    return counts, m, noct, nsup, blocks_cluster


def _chunks(blocks_cluster, nsup):
    """MM chunks (supertile, first block in supertile, nblocks<=4, cluster)."""
    chunks = []
    nb = len(blocks_cluster)
    for s in range(nsup):
        j = 0
        while j < B:
            g0 = s * B + j
            if g0 >= nb:
                break
            k = int(blocks_cluster[g0])
            run = 1
            while j + run < B and g0 + run < nb and blocks_cluster[g0 + run] == k:
                run += 1
            off = 0
            while off < run:
                g = min(4, run - off)
                chunks.append((s, j + off, g, k))
                off += g
            j += run
    return chunks


def _prune_dup_ldweights(nc):
    """Tile emits one LDWEIGHTS per matmul; with cluster-sorted blocks,
    consecutive matmuls share the stationary weights, so drop repeats
    (converted to NoOps to keep any semaphore waits/updates)."""
    import concourse.mybir as mybir

    for bb in nc.main_func.blocks:
        last_sig = None
        new = []
        for inst in bb.instructions:
            if isinstance(inst, mybir.InstLdweights):
                sig = repr(inst.ins[0])
                if sig == last_sig:
                    si = inst.sync_info
                    if si is not None and (si.on_wait or si.on_update):
                        nop = mybir.InstNoOp(
                            name=nc.get_next_instruction_name(),
                            engine=inst.engine, ins=[], outs=[],
                            sync_info=si)
                        nc.register_instruction(nop)
                        new.append(nop)
                    continue
                last_sig = sig
            new.append(inst)
        bb.instructions[:] = new


FOLD = 2              # X^2 columns folded 64 -> 32 before the matmul
DF = D // FOLD        # folded X^2 width per block
SQUARE_MODE = "split"  # "pow" (DVE tensor_scalar, rejected by ISA) | "split"


def _build_module(nsup, chunks, blocks_cluster):
    import concourse.bass as bass
    import concourse.mybir as mybir
    import concourse.tile as tile

    nc = bass.Bass()
    x_in = nc.dram_tensor("x", [nsup, P, SUPC], mybir.dt.bfloat16,
                          kind="ExternalInput")
    out = nc.dram_tensor("out", [K, D + DF], mybir.dt.float32,
                         kind="ExternalOutput")

    n_mm = len(chunks)
    per_s = {}
    for ch in chunks:
        per_s.setdefault(ch[0], []).append(ch)

    # first supertile that references a cluster >= W0K decides when the
    # big weight tile must be ready; build it early only if needed early.
    first_hi = next((i for i, k in enumerate(blocks_cluster) if k >= W0K),
                    len(blocks_cluster))
    w1_emit_s = max(0, min(first_hi // B - 2, 2))

    with ExitStack() as ctx:
        tc = ctx.enter_context(tile.TileContext(nc))
        cpool = ctx.enter_context(tc.tile_pool(name="const", bufs=1))
        xpool = ctx.enter_context(tc.tile_pool(name="x", bufs=5))
        spool = ctx.enter_context(tc.tile_pool(name="sq", bufs=3))
        ppool = ctx.enter_context(tc.tile_pool(name="psum", bufs=1, space="PSUM"))
        opool = ctx.enter_context(tc.tile_pool(name="o", bufs=1))

        ones_t = cpool.tile([P, 1], mybir.dt.bfloat16, tag="ones")
        nc.vector.memset(ones_t[:], 1.0)
        w0 = cpool.tile([P, W0K * K], mybir.dt.bfloat16, tag="w0")
        w1 = cpool.tile([P, (K - W0K) * K], mybir.dt.bfloat16, tag="w1")

        def build_w(wt, nk, base):
            nc.gpsimd.affine_select(
                out=wt[:],
                in_=ones_t[:, 0:1].broadcast_to([P, nk * K]),
                pattern=[[-1, nk], [1, K]], base=base, channel_multiplier=0,
                compare_op=mybir.AluOpType.is_equal, fill=0.0)

        build_w(w0, W0K, 0)
        w1_built = False
        if w1_emit_s == 0:
            build_w(w1, K - W0K, -W0K)
            w1_built = True

        # All matmuls run with start=False and accumulate onto this explicit
        # zero: correct whether PSUM has_written bits are stale (accumulate
        # onto 0) or clear (overwrite with the product) — and it keeps slots
        # never touched by a partial-width chunk at exactly 0 for the fold.
        # Layout: A half = 4 slots x D (segsum x), B half = 4 slots x DF
        # (segsum of dim-folded x^2).
        AW = 4 * D            # A-half width in psum
        BW = 4 * DF           # B-half width in psum
        psum_t = ppool.tile([P, AW + BW], mybir.dt.float32)
        nc.vector.memset(psum_t[:K, :], 0.0)

        mm_i = 0
        for s in range(nsup):
            xb = xpool.tile([P, SUPC + SUPC // FOLD], mybir.dt.bfloat16)
            sq = spool.tile([P, SUPC], mybir.dt.bfloat16)
            nc.sync.dma_start(out=xb[:, 0:SUPC], in_=x_in[s])
            if SQUARE_MODE == "pow" or s % 3 != 0:
                if SQUARE_MODE == "pow":
                    nc.vector.tensor_scalar(
                        out=sq[:], in0=xb[:, 0:SUPC], scalar1=2.0, scalar2=None,
                        op0=mybir.AluOpType.pow)
                else:
                    nc.vector.tensor_tensor(
                        out=sq[:], in0=xb[:, 0:SUPC], in1=xb[:, 0:SUPC],
                        op=mybir.AluOpType.mult)
            else:
                nc.scalar.activation(
                    out=sq[:], in_=xb[:, 0:SUPC],
                    func=mybir.ActivationFunctionType.Square)
            # dim-fold x^2: xf[p, j, e] = sq[p, j, e] + sq[p, j, e + DF]
            sqv = sq[:].rearrange("p (j d) -> p j d", d=D)
            xfv = xb[:, SUPC:SUPC + SUPC // FOLD].rearrange(
                "p (j e) -> p j e", e=DF)
            nc.vector.tensor_tensor(
                out=xfv, in0=sqv[:, :, 0:DF], in1=sqv[:, :, DF:D],
                op=mybir.AluOpType.add)
            if not w1_built and s >= w1_emit_s:
                build_w(w1, K - W0K, -W0K)
                w1_built = True
            for (_, j0, g, k) in per_s.get(s, []):
                lhsT = (w0[:, k * K:(k + 1) * K] if k < W0K
                        else w1[:, (k - W0K) * K:(k - W0K + 1) * K])
                nc.tensor.matmul(
                    psum_t[:K, 0:g * D], lhsT=lhsT,
                    rhs=xb[:, j0 * D:(j0 + g) * D],
                    start=False, stop=False, skip_group_check=True)
                nc.tensor.matmul(
                    psum_t[:K, AW:AW + g * DF], lhsT=lhsT,
                    rhs=xb[:, SUPC + j0 * DF:SUPC + (j0 + g) * DF],
                    start=False, stop=(mm_i == n_mm - 1),
                    skip_group_check=True)
                mm_i += 1
        if not w1_built:
            build_w(w1, K - W0K, -W0K)

        # fold the 4 block-slots of each half: S = sum_slots psum[:, slot]
        SC = AW + BW
        tmp = opool.tile([P, SC + 2 * D + 2 * DF], mybir.dt.float32, tag="tmp")
        out_sb = opool.tile([P, D + DF], mybir.dt.float32, tag="osb")
        nc.vector.tensor_copy(out=tmp[:K, 0:SC], in_=psum_t[:K, :])
        for h, (b0, w) in enumerate(((0, D), (AW, DF))):
            sc = SC + h * 2 * D
            nc.vector.tensor_tensor(
                out=tmp[:K, sc:sc + w], in0=tmp[:K, b0:b0 + w],
                in1=tmp[:K, b0 + w:b0 + 2 * w], op=mybir.AluOpType.add)
            nc.vector.tensor_tensor(
                out=tmp[:K, sc + w:sc + 2 * w], in0=tmp[:K, b0 + 2 * w:b0 + 3 * w],
                in1=tmp[:K, b0 + 3 * w:b0 + 4 * w], op=mybir.AluOpType.add)
            nc.vector.tensor_tensor(
                out=out_sb[:K, h * D:h * D + w], in0=tmp[:K, sc:sc + w],
                in1=tmp[:K, sc + w:sc + 2 * w], op=mybir.AluOpType.add)
        nc.sync.dma_start(out=out[:], in_=out_sb[:K, 0:D + DF])
    _prune_dup_ldweights(nc)
    _split_excess_waits(nc)
    return nc


def _prep_inputs(x, cls, counts, m, noct, nsup):
    """Sorted/padded per-core device arrays [nsup, P, SUPC] bf16."""
    n = x.shape[0]
    order = np.argsort(cls, kind="stable")
    sel = np.full(noct * OCT, n, np.int64)       # n -> zero row
    in_off = 0
    out_off = 0
    for k in range(K):
        nk = int(counts[k])
        sel[out_off:out_off + nk] = order[in_off:in_off + nk]
        in_off += nk
        out_off += int(m[k]) * OCT
    x_ext = np.zeros((n + 1, D), BF16)
    x_ext[:n] = x
    xg = x_ext[sel].reshape(noct, N_CORES, P, D)
    nbp = nsup * B
    in_maps = []
    for c in range(N_CORES):
        xc = np.zeros((nbp, P, D), BF16)
        xc[:noct] = xg[:, c]
        xdev = np.ascontiguousarray(
            xc.reshape(nsup, B, P, D).transpose(0, 2, 1, 3).reshape(
                nsup, P, SUPC))
        in_maps.append({"x": xdev})
    return in_maps


def prepare(x, cls):
    """Build (module, per-core inputs, counts) for the full problem."""
    counts, m, noct, nsup, blocks_cluster = _plan(cls)
    chunks = _chunks(blocks_cluster, nsup)
    nc = _build_module(nsup, chunks, blocks_cluster)
    in_maps = _prep_inputs(x, cls, counts, m, noct, nsup)
    return nc, in_maps, counts


def _dbi_from_stats(S, S2, n):
    S = S.astype(np.float64)
    Q = S2.astype(np.float64).sum(-1)
    n = n.astype(np.float64)
    counts = 1.0 + n
    A = (0.001 + S) / counts[:, None]
    segsq = Q - 2.0 * (A * S).sum(-1) + n * (A * A).sum(-1)
    Si = np.sqrt((0.001 + segsq) / counts)
    diff = A[:, None, :] - A[None, :, :]
    sumsq = (diff * diff).sum(-1)
    eye = np.eye(K, dtype=bool)
    Mij = np.sqrt(np.where(eye, 1.0, sumsq))
    Rij = np.where(eye, 0.0, (Si[:, None] + Si[None, :]) / Mij)
    return np.float32(Rij.max(axis=1).sum() / K)


def kernel(data_points: np.ndarray, clustering: np.ndarray) -> np.ndarray:
    from concourse.bass_utils import run_bass_kernel_spmd

    x = np.asarray(data_points)
    cls = np.asarray(clustering).astype(np.int64)
    assert x.shape == (N_TOTAL, D), x.shape

    nc, in_maps, counts = prepare(x, cls)
    res = run_bass_kernel_spmd(nc, in_maps, core_ids=list(range(N_CORES)))

    S = np.zeros((K, D), np.float64)
    S2 = np.zeros((K, DF), np.float64)
    for r in res.results:
        o = r["out"].astype(np.float64)
        S += o[:, :D]
        S2 += o[:, D:]
    return np.asarray(_dbi_from_stats(S, S2, counts.astype(np.float64)),
                      dtype=np.float32)
